# revision 25
# baseline (speedup 1.0000x reference)
"""Causal multi-head attention (B=2, T=2048, D=1024, H=16) on 8 TRN2 NeuronCores.

Sharding: core c = (batch c%2, head-group c//2); each core owns 4 heads
(256 dims) of one batch. Partial out-projections are summed with one fp16
ReduceScatter per 512-row q-span over each batch's 4 cores; the even/odd
replica groups [[0,2,4,6],[1,3,5,7]] measure ~30us faster than consecutive
grouping on the axon 8-core topology.

Design (vs the 352us baseline this evolved from):
  - software-pipelined score->exp->AV loop (score kt+1 issues before AV kt)
    so the PE does not serialize on the ~630ns exp.
  - causal column trimming: diagonal k-tiles only compute/exp/AV columns
    >= 128*kt - span_base; the partially-masked 128x128 block is the SAME
    upper-triangular pattern for every tile -> one 32KB tri tile replaces
    the 2MB mask load.
  - fp16 partials + one RS per span (half the collective bytes of fp32;
    fp16 keeps quantization at 2^-11 so accuracy is unchanged).
  - RS results copied to the output tensor from the gpsimd queue with a
    one-RS lag: the Tile scheduler orders instructions by cost-model
    readiness, and a collective-gated DMA placed on the scalar/sync queues
    gets scheduled mid-stream and head-of-line blocks exps/partial-writes
    for tens of us when the real RS runs slower than the model.
  - projections of span s+1 and post-processing (normalize/out-proj/RS) of
    span s-1 interleave between the attention heads of span s.
  - per-head softmax denominators from PSUM row 64 (the v-augmentation ones
    column) -> DVE reciprocal_approx_fast (needs an SBUF-resident input:
    from PSUM it returns garbage) -> bf16 rows 0/32 of a persistent [33,512]
    tile -> ONE rank-1 PE matmul per (span, dims-half) with a [33,128] 0/1
    selector broadcasts both heads' 1/den across partitions at once.
  - q/k bias evacuations on ACT via Identity+bias (same act table as Exp,
    so no 1283ns table reloads); yT/v/out evacuations on DVE.

Perf notes: the device clamps the PE clock (power/activity throttle) to
~1.2GHz and later ~0.85GHz, so the kernel is PE-column-bound: projections
98k cols + attention 2x70k + out-proj 33k + bcast 4k. Startup input DMAs
are balanced across all three DMA-capable queues (first matmul ~13-17us
in); the last span's partial writes split across sync+scalar so the final
RS fires ~4us after the last matmul. Measured 214-237us (throttle-state
dependent) vs 352-358us baseline; rel err 5.955e-3; PE idle within the
compute window is <3us.
"""

import os
import numpy as np
import ml_dtypes

BF16 = ml_dtypes.bfloat16
FP16 = np.float16

B, T, D, H = 2, 2048, 1024, 16
HD = D // H                     # 64
NCORES = 8
GROUPS = 4                      # cores per batch (tensor-parallel degree)
HL = H // GROUPS                # heads per core = 4
DL = D // GROUPS                # dims per core = 256
SCALE = HD ** -0.5

# core c = (batch c%2, head-group c//2): the even/odd replica groups measure
# ~30us faster collectives than [[0..3],[4..7]] on the axon 8-core topology
RS_GROUPS = [[0, 2, 4, 6], [1, 3, 5, 7]]


def _core_bg(c):
    return (c % 2, c // 2)


WS = [512, 512, 512, 512]       # q-span widths (sum = T)
BS = [0, 512, 1024, 1536]       # q-span base offsets
NSP = len(WS)

_CACHE = {}


def _build_program():
    import concourse.bass as bass  # noqa: F401  (registers bass machinery)
    import concourse.tile as tile
    from concourse import bacc, mybir

    f32 = mybir.dt.float32
    f32r = mybir.dt.float32r
    bf16 = mybir.dt.bfloat16
    fp16 = mybir.dt.float16
    Exp = mybir.ActivationFunctionType.Exp
    Identity = mybir.ActivationFunctionType.Identity

    nc = bacc.Bacc("TRN2", target_bir_lowering=False, debug=False,
                   num_devices=NCORES)

    xT = nc.dram_tensor("xT", [D, T], bf16, kind="ExternalInput")
    wqT = nc.dram_tensor("wqT", [D, DL], bf16, kind="ExternalInput")
    wkT = nc.dram_tensor("wkT", [D, DL], bf16, kind="ExternalInput")
    wvT = nc.dram_tensor("wvT", [D, DL], bf16, kind="ExternalInput")
    woT = nc.dram_tensor("woT", [DL, D], bf16, kind="ExternalInput")
    bqP = nc.dram_tensor("bqP", [128, 2], f32, kind="ExternalInput")
    bkP = nc.dram_tensor("bkP", [128, 2], f32, kind="ExternalInput")
    bv = nc.dram_tensor("bv", [1, DL], bf16, kind="ExternalInput")
    bo = nc.dram_tensor("bo", [1, D], bf16, kind="ExternalInput")
    trid = nc.dram_tensor("trid", [128, 128], bf16, kind="ExternalInput")
    out_ext = nc.dram_tensor("out", [T // GROUPS, D], fp16,
                             kind="ExternalOutput")

    with tile.TileContext(nc) as tc:
        with tc.tile_pool(name="main", bufs=1) as main, \
             tc.tile_pool(name="rec", bufs=8) as recp, \
             tc.tile_pool(name="at", bufs=4) as atp, \
             tc.tile_pool(name="ob", bufs=3) as obp, \
             tc.tile_pool(name="dram", bufs=1, space="DRAM") as dram, \
             tc.tile_pool(name="sc_ps", bufs=2, space="PSUM") as sc_ps, \
             tc.tile_pool(name="av_ps", bufs=2, space="PSUM") as av_ps, \
             tc.tile_pool(name="pj_ps", bufs=2, space="PSUM") as pj_ps, \
             tc.tile_pool(name="pp_ps", bufs=2, space="PSUM") as pp_ps:

            xt_s = main.tile([128, 8, T], bf16)
            wq_s = main.tile([128, 8, DL], bf16)
            wk_s = main.tile([128, 8, DL], bf16)
            wv_s = main.tile([128, 8, DL], bf16)
            wo_s = main.tile([128, 2, D], bf16)
            qT_s = main.tile([128, 2, T], bf16)
            kT_s = main.tile([128, 2, T], bf16)
            yT_s = main.tile([128, 2, T], bf16)
            v_s = main.tile([128, 16, HL * 65], bf16)
            tri_s = main.tile([128, 128], bf16)
            bq_s = main.tile([128, 2], f32)
            bk_s = main.tile([128, 2], f32)
            bv_bc = main.tile([128, DL], bf16)
            bo_bc = main.tile([128, D], bf16)
            ones_b = main.tile([1, 64], bf16)
            # two-head normalize broadcast: lhsT [33,128] selector places
            # rec row 0 on out partitions 0-63 and rec row 32 on 64-127;
            # rows 1-31 are zeroed so SBUF garbage cannot leak NaNs
            ones2 = main.tile([33, 128], bf16)
            rec2s = [main.tile([33, 512], bf16, name=f"rec2_{i}")
                     for i in range(4)]

            partials = [dram.tile([WS[i], D], fp16, name=f"partial{i}")
                        for i in range(NSP)]
            rs_outs = [dram.tile([WS[i] // GROUPS, D], fp16, name=f"rsout{i}")
                       for i in range(NSP)]

            # ---- input DMAs: the first q-projection group needs all of wq
            # plus x[:, :, 0:256]; split those across all three DMA-capable
            # queues so the PE starts ~12us in instead of ~18us
            wq_r = wqT[:].rearrange("(c p) n -> p c n", p=128)
            nc.scalar.dma_start(out=wq_s[:, 0:4, :], in_=wq_r[:, 0:4, :])
            nc.scalar.dma_start(out=bq_s, in_=bqP[:])
            nc.scalar.dma_start(out=bk_s, in_=bkP[:])
            wk_r = wkT[:].rearrange("(c p) n -> p c n", p=128)
            nc.scalar.dma_start(out=wk_s, in_=wk_r)
            # gpsimd queue: first quarter of x (parallel with wq halves)
            xT_r = xT[:].rearrange("(c p) t -> p c t", p=128)
            nc.gpsimd.dma_start(out=xt_s[:, :, 0:256], in_=xT_r[:, :, 0:256])
            # sync queue: other wq half, rest of x piece 0, then weights
            # interleaved with the remaining x pieces in consumption order
            nc.sync.dma_start(out=wq_s[:, 4:8, :], in_=wq_r[:, 4:8, :])
            nc.sync.dma_start(out=xt_s[:, :, 256:512], in_=xT_r[:, :, 256:512])
            wv_r = wvT[:].rearrange("(c p) n -> p c n", p=128)
            nc.sync.dma_start(out=wv_s, in_=wv_r)
            nc.sync.dma_start(out=bv_bc, in_=bv[:].to_broadcast([128, DL]))
            nc.sync.dma_start(out=tri_s, in_=trid[:])
            nc.sync.dma_start(out=xt_s[:, :, 512:1024], in_=xT_r[:, :, 512:1024])
            wo_r = woT[:].rearrange("(c p) n -> p c n", p=128)
            nc.sync.dma_start(out=wo_s, in_=wo_r)
            nc.sync.dma_start(out=bo_bc, in_=bo[:].to_broadcast([128, D]))
            for lo, hi in ((1024, 1536), (1536, 2048)):
                nc.sync.dma_start(out=xt_s[:, :, lo:hi], in_=xT_r[:, :, lo:hi])

            nc.gpsimd.memset(ones_b, 1.0)
            nc.gpsimd.memset(ones2, 0.0)
            nc.gpsimd.memset(ones2[0:1, 0:64], 1.0)
            nc.gpsimd.memset(ones2[32:33, 64:128], 1.0)
            for r2 in rec2s:
                nc.gpsimd.memset(r2, 0.0)
            nc.vector.memset(v_s, 1.0)   # ones column at index 64 per head

            # ---------------- emission helpers ----------------
            heads_ps = {}   # (qs, h) -> av psum tile awaiting evacuation

            def proj_q(s, w_s, b_s, dst, mcs=(0, 1), nsplit=1):
                bb, ww = BS[s], WS[s]
                for mc in mcs:
                    for sp in range(nsplit):
                        w0 = ww // nsplit
                        lo = bb + sp * w0
                        ps = pj_ps.tile([128, 512], f32, tag="pj")
                        for kc in range(8):
                            nc.tensor.matmul(
                                ps[:, :w0],
                                lhsT=w_s[:, kc, mc * 128:(mc + 1) * 128],
                                rhs=xt_s[:, kc, lo:lo + w0],
                                start=(kc == 0), stop=(kc == 7))
                        nc.scalar.activation(
                            dst[:, mc, lo:lo + w0], ps[:, :w0], Identity,
                            bias=b_s[:, mc:mc + 1])

            def proj_v(s, mts=None):
                if mts is None:
                    mts = range(BS[s] // 128, (BS[s] + WS[s]) // 128)
                for mt in mts:
                    ps = pj_ps.tile([128, 512], f32, tag="pj")
                    for kc in range(8):
                        nc.tensor.matmul(
                            ps[:, :DL],
                            lhsT=xt_s[:, kc, mt * 128:(mt + 1) * 128],
                            rhs=wv_s[:, kc, :],
                            start=(kc == 0), stop=(kc == 7))
                    nc.vector.tensor_add(
                        v_s[:, mt, :].rearrange(
                            "p (h d) -> p h d", d=65)[:, :, 0:64],
                        ps[:, :DL].rearrange("p (h d) -> p h d", d=64),
                        bv_bc.rearrange("p (h d) -> p h d", d=64))

            def attn_head(qs, h):
                bb, ww = BS[qs], WS[qs]
                mc, r0 = divmod(h, 2)
                r0 *= 64
                qv = qT_s[r0:r0 + 64, mc, bb:bb + ww]
                nkt = (bb + ww) // 128
                nfull = bb // 128
                av_t = av_ps.tile([65, 512], f32, tag="av")

                def score(kt):
                    c0 = max(0, 128 * kt - bb)
                    sc_t = sc_ps.tile([128, 512], f32, tag="sc")
                    nc.tensor.matmul(
                        sc_t[:, c0:ww],
                        lhsT=kT_s[r0:r0 + 64, mc, kt * 128:(kt + 1) * 128],
                        rhs=qv[:, c0:ww], start=True, stop=True)
                    return sc_t, c0

                nxt = score(0)
                for kt in range(nkt):
                    sc_t, c0 = nxt
                    if kt + 1 < nkt:
                        nxt = score(kt + 1)  # PE runs ahead of the exp
                    at = atp.tile([128, 512], bf16, tag="at")
                    nc.scalar.activation(at[:, c0:ww], sc_t[:, c0:ww], Exp)
                    if kt >= nfull:  # diagonal tile: mask its 128-col block
                        nc.vector.tensor_mul(
                            at[:, c0:c0 + 128], at[:, c0:c0 + 128], tri_s)
                    nc.tensor.matmul(
                        av_t[:, c0:ww], lhsT=v_s[:, kt, h * 65:(h + 1) * 65],
                        rhs=at[:, c0:ww],
                        start=(kt == 0), stop=(kt == nkt - 1))
                heads_ps[(qs, h)] = av_t

            def evac(qs, h):
                bb, ww = BS[qs], WS[qs]
                mc, r0 = divmod(h, 2)
                r0 *= 64
                av_t = heads_ps.pop((qs, h))
                nc.vector.tensor_copy(
                    yT_s[r0:r0 + 64, mc, bb:bb + ww], av_t[0:64, :ww])
                den = recp.tile([1, 512], f32, tag="den")
                nc.vector.tensor_copy(den[:, :ww], av_t[64:65, :ww])
                rec = recp.tile([1, 512], f32, tag="rec")
                nc.vector.reciprocal_approx_fast(rec[:, :ww], den[:, :ww])
                r2 = rec2s[(qs % 2) * 2 + mc]
                row = (h % 2) * 32
                nc.vector.tensor_copy(r2[row:row + 1, :ww], rec[:, :ww])

            def post_norm(qs):
                bb, ww = BS[qs], WS[qs]
                for mc in range(2):
                    r2 = rec2s[(qs % 2) * 2 + mc]
                    rb = pp_ps.tile([128, 512], f32, tag="pp")
                    nc.tensor.matmul(rb[:, :ww], lhsT=ones2,
                                     rhs=r2[:, :ww],
                                     start=True, stop=True)
                    yv = yT_s[:, mc, bb:bb + ww]
                    nc.vector.tensor_mul(yv, yv, rb[:, :ww])

            def post_qt(qs, lq):
                qt = BS[qs] // 128 + lq
                ob = obp.tile([128, D], fp16, tag="ob")
                for ns in range(2):
                    po = pp_ps.tile([128, 512], f32, tag="pp")
                    for kc in range(2):
                        nc.tensor.matmul(
                            po,
                            lhsT=yT_s[:, kc, qt * 128:(qt + 1) * 128],
                            rhs=wo_s[:, kc, ns * 512:(ns + 1) * 512],
                            start=(kc == 0), stop=(kc == 1))
                    nc.vector.tensor_add(
                        ob[:, ns * 512:(ns + 1) * 512], po,
                        bo_bc[:, ns * 512:(ns + 1) * 512])
                # last span: exps are done, so the scalar queue is safe to
                # share the partial writes -> final RS triggers sooner
                eng = nc.scalar if (qs == NSP - 1 and lq % 2) else nc.sync
                eng.dma_start(
                    out=partials[qs][lq * 128:(lq + 1) * 128, :], in_=ob)

            def post_rs(qs):
                nc.gpsimd.collective_compute(
                    "ReduceScatter", mybir.AluOpType.add,
                    replica_groups=RS_GROUPS,
                    ins=[partials[qs][:].opt()],
                    outs=[rs_outs[qs][:].opt()])
                # copy of the PREVIOUS span's RS result: it waits on that
                # (long-done) RS only, so it cannot stall this queue; the
                # gpsimd queue carries nothing compute-critical anyway
                if qs >= 1:
                    out_copy(qs - 1)

            def out_copy(qs):
                bb, ww = BS[qs], WS[qs]
                nc.gpsimd.dma_start(out=out_ext[bb // 4:(bb + ww) // 4, :],
                                    in_=rs_outs[qs][:])

            # ---------------- program ----------------
            # span-0 projections ordered so heads 0/1 (dims chunk 0) can
            # start their exps as early as possible
            proj_q(0, wq_s, bq_s, qT_s, mcs=(0,), nsplit=2)
            proj_q(0, wk_s, bk_s, kT_s, mcs=(0,), nsplit=2)
            proj_v(0)
            proj_q(0, wq_s, bq_s, qT_s, mcs=(1,), nsplit=2)
            proj_q(0, wk_s, bk_s, kT_s, mcs=(1,), nsplit=2)

            for qs in range(NSP):
                nqt = WS[qs] // 128
                prev = qs - 1
                if prev >= 0:
                    post_norm(prev)
                attn_head(qs, 0)
                attn_head(qs, 1)
                evac(qs, 0)
                if prev >= 0:
                    for lq in range(0, min(2, WS[prev] // 128)):
                        post_qt(prev, lq)
                if qs + 1 < NSP:
                    proj_q(qs + 1, wq_s, bq_s, qT_s)
                attn_head(qs, 2)
                evac(qs, 1)
                if prev >= 0:
                    for lq in range(2, WS[prev] // 128):
                        post_qt(prev, lq)
                    post_rs(prev)
                if qs + 1 < NSP:
                    proj_q(qs + 1, wk_s, bk_s, kT_s)
                attn_head(qs, 3)
                evac(qs, 2)
                if qs + 1 < NSP:
                    proj_v(qs + 1)
                evac(qs, 3)

            last = NSP - 1
            post_norm(last)
            for lq in range(WS[last] // 128):
                post_qt(last, lq)
            post_rs(last)
            out_copy(NSP - 1)

    nc.compile()
    return nc


def _get_program():
    if "nc" not in _CACHE:
        _CACHE["nc"] = _build_program()
    return _CACHE["nc"]


def _make_in_maps(x, mask, Wq, bq, Wk, bk, Wv, bv, Wo, bo):
    x = np.asarray(x, np.float32)
    Wq = np.asarray(Wq, np.float32)
    Wk = np.asarray(Wk, np.float32)
    Wv = np.asarray(Wv, np.float32)
    Wo = np.asarray(Wo, np.float32)
    bq = np.asarray(bq, np.float32)
    bk = np.asarray(bk, np.float32)
    bv = np.asarray(bv, np.float32)
    bo = np.asarray(bo, np.float32)

    tri = np.triu(np.ones((128, 128), np.float32)).astype(BF16)
    zeros_bo = np.zeros((1, D), np.float32)
    in_maps = []
    xTb = {b: np.ascontiguousarray(x[b].T) for b in range(B)}
    for c in range(NCORES):
        b, g = _core_bg(c)
        sl = slice(g * DL, (g + 1) * DL)
        in_maps.append({
            "xT": xTb[b].astype(BF16),
            "wqT": np.ascontiguousarray((Wq[sl] * SCALE).T).astype(BF16),
            "wkT": np.ascontiguousarray(Wk[sl].T).astype(BF16),
            "wvT": np.ascontiguousarray(Wv[sl].T).astype(BF16),
            "woT": np.ascontiguousarray(Wo[:, sl].T).astype(BF16),
            "bqP": np.ascontiguousarray((bq[sl] * SCALE).reshape(2, 128).T),
            "bkP": np.ascontiguousarray(bk[sl].reshape(2, 128).T),
            "bv": bv[sl].reshape(1, DL).astype(BF16),
            "bo": (bo.reshape(1, D) if g == 0 else zeros_bo).astype(BF16),
            "trid": tri,
        })
    return in_maps


def _capture_profile(nc, in_maps, tmpdir):
    """Run with NTFF capture and process the profile ourselves (the stock
    trace path can't handle the duplicate-executable NTFFs the axon relay
    produces). Returns (results, exec_time_ns|None)."""
    import glob
    import json
    import re
    import subprocess
    from trn_agent_boot.trn_boot import _ntff_profile_via_ctypes
    from concourse import bass2jax

    hook = _ntff_profile_via_ctypes("/opt/axon/libaxon_pjrt.so")
    if hook is None:
        raise RuntimeError("libaxon_pjrt.so lacks NTFF profile symbols")
    os.makedirs(tmpdir, exist_ok=True)
    with hook(tmpdir, [0]):
        results = bass2jax.run_bass_via_pjrt(nc, in_maps, n_cores=NCORES)

    ntffs = glob.glob(os.path.join(tmpdir, "*_body*-device*.ntff"))
    best, best_id = None, -1
    for f in ntffs:
        m = re.search(r"executable(\d+)-device000000", f)
        if m and int(m.group(1)) > best_id:
            best_id, best = int(m.group(1)), f
    if best is None:
        raise RuntimeError(f"no NTFF produced in {tmpdir}")
    neff = re.sub(r"-device\d+-execution-\d+\.ntff$", ".neff", best)
    out_json = os.path.join(tmpdir, "prof.json")
    subprocess.check_call(
        ["neuron-profile", "view", "--ignore-nc-buf-usage", "-s", best,
         "-n", neff, "--output-format=json", f"--output-file={out_json}"],
        cwd=tmpdir)
    summary = json.load(open(out_json))["summary"][0]
    return results, int(summary["total_time"] * 1e9)


def kernel(x, mask, Wq, bq, Wk, bk, Wv, bv, Wo, bo):
    from concourse import bass_utils

    in_maps = _make_in_maps(x, mask, Wq, bq, Wk, bk, Wv, bv, Wo, bo)
    nc = _get_program()

    trace = bool(int(os.environ.get("MHA_TRACE", "0")))
    tmpdir = os.environ.get("MHA_TRACE_DIR") or None
    results = None
    if trace and tmpdir:
        try:
            results, exec_ns = _capture_profile(nc, in_maps, tmpdir)
            _CACHE["last_exec_time_ns"] = exec_ns
        except Exception as e:  # profiling is best-effort
            print(f"profiling unavailable: {type(e).__name__}: {e}")
            results = None
    if results is None:
        results = bass_utils.run_bass_kernel_spmd(
            nc, in_maps, core_ids=list(range(NCORES))).results
        _CACHE.setdefault("last_exec_time_ns", None)

    out = np.empty((B, T, D), np.float32)
    for c in range(NCORES):
        b, g = _core_bg(c)
        o = np.asarray(results[c]["out"], np.float32)
        for qs in range(NSP):
            bb, wq4 = BS[qs], WS[qs] // 4
            out[b, bb + g * wq4: bb + (g + 1) * wq4] = \
                o[bb // 4: bb // 4 + wq4]
    return out


# revision 26
# speedup vs baseline: 1.0413x; 1.0413x over previous
"""Causal multi-head attention (B=2, T=2048, D=1024, H=16) on 8 TRN2 NeuronCores.

Sharding: core c = (batch c%2, head-group c//2); each core owns 4 heads
(256 dims) of one batch. Partial out-projections are summed with one fp16
ReduceScatter per 512-row q-span over each batch's 4 cores; the even/odd
replica groups [[0,2,4,6],[1,3,5,7]] measure ~30us faster than consecutive
grouping on the axon 8-core topology.

Design (vs the 352us baseline this evolved from):
  - software-pipelined score->exp->AV loop (score kt+1 issues before AV kt)
    so the PE does not serialize on the ~630ns exp.
  - causal column trimming: diagonal k-tiles only compute/exp/AV columns
    >= 128*kt - span_base; the partially-masked 128x128 block is the SAME
    upper-triangular pattern for every tile -> one 32KB tri tile replaces
    the 2MB mask load.
  - fp16 partials + one RS per span (half the collective bytes of fp32;
    fp16 keeps quantization at 2^-11 so accuracy is unchanged).
  - RS results copied to the output tensor from the gpsimd queue with a
    one-RS lag: the Tile scheduler orders instructions by cost-model
    readiness, and a collective-gated DMA placed on the scalar/sync queues
    gets scheduled mid-stream and head-of-line blocks exps/partial-writes
    for tens of us when the real RS runs slower than the model.
  - projections of span s+1 and post-processing (normalize/out-proj/RS) of
    span s-1 interleave between the attention heads of span s.
  - per-head softmax denominators from PSUM row 64 (the v-augmentation ones
    column) -> DVE reciprocal_approx_fast (needs an SBUF-resident input:
    from PSUM it returns garbage) -> bf16 rows 0/32 of a persistent [33,512]
    tile -> ONE rank-1 PE matmul per (span, dims-half) with a [33,128] 0/1
    selector broadcasts both heads' 1/den across partitions at once.
  - q/k bias evacuations on ACT via Identity+bias (same act table as Exp,
    so no 1283ns table reloads); yT/v/out evacuations on DVE.

Perf notes: the device clamps the PE clock (power/activity throttle) to
~1.2GHz and later ~0.85GHz, so the kernel is PE-column-bound: projections
98k cols + attention 2x70k + out-proj 33k + bcast 4k. Startup input DMAs
are balanced across all three DMA-capable queues (first matmul ~13-17us
in); the last span's partial writes split across sync+scalar so the final
RS fires ~4us after the last matmul. Measured 214-237us (throttle-state
dependent) vs 352-358us baseline; rel err 5.955e-3; PE idle within the
compute window is <3us.
"""

import os
import numpy as np
import ml_dtypes

BF16 = ml_dtypes.bfloat16
FP16 = np.float16

B, T, D, H = 2, 2048, 1024, 16
HD = D // H                     # 64
NCORES = 8
GROUPS = 4                      # cores per batch (tensor-parallel degree)
HL = H // GROUPS                # heads per core = 4
DL = D // GROUPS                # dims per core = 256
SCALE = HD ** -0.5

# core c = (batch c%2, head-group c//2): the even/odd replica groups measure
# ~30us faster collectives than [[0..3],[4..7]] on the axon 8-core topology
RS_GROUPS = [[0, 2, 4, 6], [1, 3, 5, 7]]


def _core_bg(c):
    return (c % 2, c // 2)


WS = [512, 512, 512, 512]       # q-span widths (sum = T)
BS = [0, 512, 1024, 1536]       # q-span base offsets
NSP = len(WS)

_CACHE = {}


def _build_program():
    import concourse.bass as bass  # noqa: F401  (registers bass machinery)
    import concourse.tile as tile
    from concourse import bacc, mybir

    f32 = mybir.dt.float32
    f32r = mybir.dt.float32r
    bf16 = mybir.dt.bfloat16
    fp16 = mybir.dt.float16
    Exp = mybir.ActivationFunctionType.Exp
    Identity = mybir.ActivationFunctionType.Identity

    nc = bacc.Bacc("TRN2", target_bir_lowering=False, debug=False,
                   num_devices=NCORES)

    xT = nc.dram_tensor("xT", [D, T], bf16, kind="ExternalInput")
    wqT = nc.dram_tensor("wqT", [D, DL], bf16, kind="ExternalInput")
    wkT = nc.dram_tensor("wkT", [D, DL], bf16, kind="ExternalInput")
    wvT = nc.dram_tensor("wvT", [D, DL], bf16, kind="ExternalInput")
    woT = nc.dram_tensor("woT", [DL, D], bf16, kind="ExternalInput")
    bqP = nc.dram_tensor("bqP", [128, 2], f32, kind="ExternalInput")
    bkP = nc.dram_tensor("bkP", [128, 2], f32, kind="ExternalInput")
    bv = nc.dram_tensor("bv", [1, DL], bf16, kind="ExternalInput")
    bo = nc.dram_tensor("bo", [1, D], bf16, kind="ExternalInput")
    trid = nc.dram_tensor("trid", [128, 128], bf16, kind="ExternalInput")
    out_ext = nc.dram_tensor("out", [T // GROUPS, D], fp16,
                             kind="ExternalOutput")

    with tile.TileContext(nc) as tc:
        with tc.tile_pool(name="main", bufs=1) as main, \
             tc.tile_pool(name="rec", bufs=8) as recp, \
             tc.tile_pool(name="at", bufs=4) as atp, \
             tc.tile_pool(name="ob", bufs=3) as obp, \
             tc.tile_pool(name="dram", bufs=1, space="DRAM") as dram, \
             tc.tile_pool(name="sc_ps", bufs=2, space="PSUM") as sc_ps, \
             tc.tile_pool(name="av_ps", bufs=2, space="PSUM") as av_ps, \
             tc.tile_pool(name="pj_ps", bufs=2, space="PSUM") as pj_ps, \
             tc.tile_pool(name="pp_ps", bufs=2, space="PSUM") as pp_ps:

            xt_s = main.tile([128, 8, T], bf16)
            wq_s = main.tile([128, 8, DL], bf16)
            wk_s = main.tile([128, 8, DL], bf16)
            wv_s = main.tile([128, 8, DL], bf16)
            wo_s = main.tile([128, 2, D], bf16)
            qT_s = main.tile([128, 2, T], bf16)
            kT_s = main.tile([128, 2, T], bf16)
            yT_s = main.tile([128, 2, T], bf16)
            v_s = main.tile([128, 16, HL * 65], bf16)
            tri_s = main.tile([128, 128], bf16)
            bq_s = main.tile([128, 2], f32)
            bk_s = main.tile([128, 2], f32)
            bv_bc = main.tile([128, DL], bf16)
            bo_bc = main.tile([128, D], bf16)
            ones_b = main.tile([1, 64], bf16)
            # two-head normalize broadcast: lhsT [33,128] selector places
            # rec row 0 on out partitions 0-63 and rec row 32 on 64-127;
            # rows 1-31 are zeroed so SBUF garbage cannot leak NaNs
            ones2 = main.tile([33, 128], bf16)
            rec2s = [main.tile([33, 512], bf16, name=f"rec2_{i}")
                     for i in range(4)]

            partials = [dram.tile([WS[i], D], fp16, name=f"partial{i}")
                        for i in range(NSP)]
            rs_outs = [dram.tile([WS[i] // GROUPS, D], fp16, name=f"rsout{i}")
                       for i in range(NSP)]

            # ---- input DMAs: the first q-projection group needs all of wq
            # plus x[:, :, 0:256]; split those across all three DMA-capable
            # queues so the PE starts ~12us in instead of ~18us
            wq_r = wqT[:].rearrange("(c p) n -> p c n", p=128)
            nc.scalar.dma_start(out=wq_s[:, 0:4, :], in_=wq_r[:, 0:4, :])
            nc.scalar.dma_start(out=bq_s, in_=bqP[:])
            nc.scalar.dma_start(out=bk_s, in_=bkP[:])
            wk_r = wkT[:].rearrange("(c p) n -> p c n", p=128)
            nc.scalar.dma_start(out=wk_s, in_=wk_r)
            # gpsimd queue: first quarter of x (parallel with wq halves)
            xT_r = xT[:].rearrange("(c p) t -> p c t", p=128)
            nc.gpsimd.dma_start(out=xt_s[:, :, 0:256], in_=xT_r[:, :, 0:256])
            # sync queue: other wq half, rest of x piece 0, then weights
            # interleaved with the remaining x pieces in consumption order
            nc.sync.dma_start(out=wq_s[:, 4:8, :], in_=wq_r[:, 4:8, :])
            nc.sync.dma_start(out=xt_s[:, :, 256:512], in_=xT_r[:, :, 256:512])
            wv_r = wvT[:].rearrange("(c p) n -> p c n", p=128)
            nc.sync.dma_start(out=wv_s, in_=wv_r)
            nc.sync.dma_start(out=bv_bc, in_=bv[:].to_broadcast([128, DL]))
            nc.sync.dma_start(out=tri_s, in_=trid[:])
            nc.sync.dma_start(out=xt_s[:, :, 512:1024], in_=xT_r[:, :, 512:1024])
            wo_r = woT[:].rearrange("(c p) n -> p c n", p=128)
            nc.sync.dma_start(out=wo_s, in_=wo_r)
            nc.sync.dma_start(out=bo_bc, in_=bo[:].to_broadcast([128, D]))
            for lo, hi in ((1024, 1536), (1536, 2048)):
                nc.sync.dma_start(out=xt_s[:, :, lo:hi], in_=xT_r[:, :, lo:hi])

            nc.gpsimd.memset(ones_b, 1.0)
            nc.gpsimd.memset(ones2, 0.0)
            nc.gpsimd.memset(ones2[0:1, 0:64], 1.0)
            nc.gpsimd.memset(ones2[32:33, 64:128], 1.0)
            for r2 in rec2s:
                nc.gpsimd.memset(r2, 0.0)
            nc.vector.memset(v_s, 1.0)   # ones column at index 64 per head

            # ---------------- emission helpers ----------------
            heads_ps = {}   # (qs, h) -> av psum tile awaiting evacuation

            def proj_q(s, w_s, b_s, dst, mcs=(0, 1), nsplit=1):
                bb, ww = BS[s], WS[s]
                for mc in mcs:
                    for sp in range(nsplit):
                        w0 = ww // nsplit
                        lo = bb + sp * w0
                        ps = pj_ps.tile([128, 512], f32, tag="pj")
                        for kc in range(8):
                            nc.tensor.matmul(
                                ps[:, :w0],
                                lhsT=w_s[:, kc, mc * 128:(mc + 1) * 128],
                                rhs=xt_s[:, kc, lo:lo + w0],
                                start=(kc == 0), stop=(kc == 7))
                        nc.scalar.activation(
                            dst[:, mc, lo:lo + w0], ps[:, :w0], Identity,
                            bias=b_s[:, mc:mc + 1])

            def proj_v(s, mts=None):
                if mts is None:
                    mts = range(BS[s] // 128, (BS[s] + WS[s]) // 128)
                for mt in mts:
                    ps = pj_ps.tile([128, 512], f32, tag="pj")
                    for kc in range(8):
                        nc.tensor.matmul(
                            ps[:, :DL],
                            lhsT=xt_s[:, kc, mt * 128:(mt + 1) * 128],
                            rhs=wv_s[:, kc, :],
                            start=(kc == 0), stop=(kc == 7))
                    nc.vector.tensor_add(
                        v_s[:, mt, :].rearrange(
                            "p (h d) -> p h d", d=65)[:, :, 0:64],
                        ps[:, :DL].rearrange("p (h d) -> p h d", d=64),
                        bv_bc.rearrange("p (h d) -> p h d", d=64))

            def attn_head(qs, h):
                bb, ww = BS[qs], WS[qs]
                mc, r0 = divmod(h, 2)
                r0 *= 64
                qv = qT_s[r0:r0 + 64, mc, bb:bb + ww]
                nkt = (bb + ww) // 128
                nfull = bb // 128
                av_t = av_ps.tile([65, 512], f32, tag="av")

                def score(kt):
                    c0 = max(0, 128 * kt - bb)
                    sc_t = sc_ps.tile([128, 512], f32, tag="sc")
                    nc.tensor.matmul(
                        sc_t[:, c0:ww],
                        lhsT=kT_s[r0:r0 + 64, mc, kt * 128:(kt + 1) * 128],
                        rhs=qv[:, c0:ww], start=True, stop=True)
                    return sc_t, c0

                nxt = score(0)
                for kt in range(nkt):
                    sc_t, c0 = nxt
                    if kt + 1 < nkt:
                        nxt = score(kt + 1)  # PE runs ahead of the exp
                    at = atp.tile([128, 512], bf16, tag="at")
                    nc.scalar.activation(at[:, c0:ww], sc_t[:, c0:ww], Exp)
                    if kt >= nfull:  # diagonal tile: mask its 128-col block
                        nc.vector.tensor_mul(
                            at[:, c0:c0 + 128], at[:, c0:c0 + 128], tri_s)
                    nc.tensor.matmul(
                        av_t[:, c0:ww], lhsT=v_s[:, kt, h * 65:(h + 1) * 65],
                        rhs=at[:, c0:ww],
                        start=(kt == 0), stop=(kt == nkt - 1))
                heads_ps[(qs, h)] = av_t

            def evac(qs, h):
                bb, ww = BS[qs], WS[qs]
                mc, r0 = divmod(h, 2)
                r0 *= 64
                av_t = heads_ps.pop((qs, h))
                nc.vector.tensor_copy(
                    yT_s[r0:r0 + 64, mc, bb:bb + ww], av_t[0:64, :ww])
                den = recp.tile([1, 512], f32, tag="den")
                nc.vector.tensor_copy(den[:, :ww], av_t[64:65, :ww])
                rec = recp.tile([1, 512], f32, tag="rec")
                nc.vector.reciprocal_approx_fast(rec[:, :ww], den[:, :ww])
                r2 = rec2s[(qs % 2) * 2 + mc]
                row = (h % 2) * 32
                nc.vector.tensor_copy(r2[row:row + 1, :ww], rec[:, :ww])

            def post_norm(qs):
                bb, ww = BS[qs], WS[qs]
                for mc in range(2):
                    r2 = rec2s[(qs % 2) * 2 + mc]
                    rb = pp_ps.tile([128, 512], f32, tag="pp")
                    nc.tensor.matmul(rb[:, :ww], lhsT=ones2,
                                     rhs=r2[:, :ww],
                                     start=True, stop=True)
                    yv = yT_s[:, mc, bb:bb + ww]
                    nc.vector.tensor_mul(yv, yv, rb[:, :ww])

            def post_qt(qs, lq):
                qt = BS[qs] // 128 + lq
                ob = obp.tile([128, D], fp16, tag="ob")
                for ns in range(2):
                    po = pp_ps.tile([128, 512], f32, tag="pp")
                    for kc in range(2):
                        nc.tensor.matmul(
                            po,
                            lhsT=yT_s[:, kc, qt * 128:(qt + 1) * 128],
                            rhs=wo_s[:, kc, ns * 512:(ns + 1) * 512],
                            start=(kc == 0), stop=(kc == 1))
                    nc.vector.tensor_add(
                        ob[:, ns * 512:(ns + 1) * 512], po,
                        bo_bc[:, ns * 512:(ns + 1) * 512])
                if qs == NSP - 1:
                    # last span: exps are done, so the scalar queue can share
                    # the partial writes; row-halves in parallel across both
                    # queues get the final RS triggered ~2us sooner
                    nc.sync.dma_start(
                        out=partials[qs][lq * 128:lq * 128 + 64, :],
                        in_=ob[0:64, :])
                    nc.scalar.dma_start(
                        out=partials[qs][lq * 128 + 64:(lq + 1) * 128, :],
                        in_=ob[64:128, :])
                else:
                    nc.sync.dma_start(
                        out=partials[qs][lq * 128:(lq + 1) * 128, :], in_=ob)

            def post_rs(qs):
                nc.gpsimd.collective_compute(
                    "ReduceScatter", mybir.AluOpType.add,
                    replica_groups=RS_GROUPS,
                    ins=[partials[qs][:].opt()],
                    outs=[rs_outs[qs][:].opt()])
                # copy of the PREVIOUS span's RS result: it waits on that
                # (long-done) RS only, so it cannot stall this queue; the
                # gpsimd queue carries nothing compute-critical anyway
                if qs >= 1:
                    out_copy(qs - 1)

            def out_copy(qs):
                bb, ww = BS[qs], WS[qs]
                nc.gpsimd.dma_start(out=out_ext[bb // 4:(bb + ww) // 4, :],
                                    in_=rs_outs[qs][:])

            # ---------------- program ----------------
            # span-0 projections ordered so heads 0/1 (dims chunk 0) can
            # start their exps as early as possible
            proj_q(0, wq_s, bq_s, qT_s, mcs=(0,), nsplit=2)
            proj_q(0, wk_s, bk_s, kT_s, mcs=(0,), nsplit=2)
            proj_v(0)
            proj_q(0, wq_s, bq_s, qT_s, mcs=(1,), nsplit=2)
            proj_q(0, wk_s, bk_s, kT_s, mcs=(1,), nsplit=2)

            for qs in range(NSP):
                nqt = WS[qs] // 128
                prev = qs - 1
                if prev >= 0:
                    post_norm(prev)
                attn_head(qs, 0)
                attn_head(qs, 1)
                evac(qs, 0)
                if prev >= 0:
                    for lq in range(0, min(2, WS[prev] // 128)):
                        post_qt(prev, lq)
                if qs + 1 < NSP:
                    proj_q(qs + 1, wq_s, bq_s, qT_s)
                attn_head(qs, 2)
                evac(qs, 1)
                if prev >= 0:
                    for lq in range(2, WS[prev] // 128):
                        post_qt(prev, lq)
                    post_rs(prev)
                if qs + 1 < NSP:
                    proj_q(qs + 1, wk_s, bk_s, kT_s)
                attn_head(qs, 3)
                evac(qs, 2)
                if qs + 1 < NSP:
                    proj_v(qs + 1)
                evac(qs, 3)

            last = NSP - 1
            post_norm(last)
            for lq in range(WS[last] // 128):
                post_qt(last, lq)
            post_rs(last)
            out_copy(NSP - 1)

    nc.compile()
    return nc


def _get_program():
    if "nc" not in _CACHE:
        _CACHE["nc"] = _build_program()
    return _CACHE["nc"]


def _make_in_maps(x, mask, Wq, bq, Wk, bk, Wv, bv, Wo, bo):
    x = np.asarray(x, np.float32)
    Wq = np.asarray(Wq, np.float32)
    Wk = np.asarray(Wk, np.float32)
    Wv = np.asarray(Wv, np.float32)
    Wo = np.asarray(Wo, np.float32)
    bq = np.asarray(bq, np.float32)
    bk = np.asarray(bk, np.float32)
    bv = np.asarray(bv, np.float32)
    bo = np.asarray(bo, np.float32)

    tri = np.triu(np.ones((128, 128), np.float32)).astype(BF16)
    zeros_bo = np.zeros((1, D), np.float32)
    in_maps = []
    xTb = {b: np.ascontiguousarray(x[b].T) for b in range(B)}
    for c in range(NCORES):
        b, g = _core_bg(c)
        sl = slice(g * DL, (g + 1) * DL)
        in_maps.append({
            "xT": xTb[b].astype(BF16),
            "wqT": np.ascontiguousarray((Wq[sl] * SCALE).T).astype(BF16),
            "wkT": np.ascontiguousarray(Wk[sl].T).astype(BF16),
            "wvT": np.ascontiguousarray(Wv[sl].T).astype(BF16),
            "woT": np.ascontiguousarray(Wo[:, sl].T).astype(BF16),
            "bqP": np.ascontiguousarray((bq[sl] * SCALE).reshape(2, 128).T),
            "bkP": np.ascontiguousarray(bk[sl].reshape(2, 128).T),
            "bv": bv[sl].reshape(1, DL).astype(BF16),
            "bo": (bo.reshape(1, D) if g == 0 else zeros_bo).astype(BF16),
            "trid": tri,
        })
    return in_maps


def _capture_profile(nc, in_maps, tmpdir):
    """Run with NTFF capture and process the profile ourselves (the stock
    trace path can't handle the duplicate-executable NTFFs the axon relay
    produces). Returns (results, exec_time_ns|None)."""
    import glob
    import json
    import re
    import subprocess
    from trn_agent_boot.trn_boot import _ntff_profile_via_ctypes
    from concourse import bass2jax

    hook = _ntff_profile_via_ctypes("/opt/axon/libaxon_pjrt.so")
    if hook is None:
        raise RuntimeError("libaxon_pjrt.so lacks NTFF profile symbols")
    os.makedirs(tmpdir, exist_ok=True)
    with hook(tmpdir, [0]):
        results = bass2jax.run_bass_via_pjrt(nc, in_maps, n_cores=NCORES)

    ntffs = glob.glob(os.path.join(tmpdir, "*_body*-device*.ntff"))
    best, best_id = None, -1
    for f in ntffs:
        m = re.search(r"executable(\d+)-device000000", f)
        if m and int(m.group(1)) > best_id:
            best_id, best = int(m.group(1)), f
    if best is None:
        raise RuntimeError(f"no NTFF produced in {tmpdir}")
    neff = re.sub(r"-device\d+-execution-\d+\.ntff$", ".neff", best)
    out_json = os.path.join(tmpdir, "prof.json")
    subprocess.check_call(
        ["neuron-profile", "view", "--ignore-nc-buf-usage", "-s", best,
         "-n", neff, "--output-format=json", f"--output-file={out_json}"],
        cwd=tmpdir)
    summary = json.load(open(out_json))["summary"][0]
    return results, int(summary["total_time"] * 1e9)


def kernel(x, mask, Wq, bq, Wk, bk, Wv, bv, Wo, bo):
    from concourse import bass_utils

    in_maps = _make_in_maps(x, mask, Wq, bq, Wk, bk, Wv, bv, Wo, bo)
    nc = _get_program()

    trace = bool(int(os.environ.get("MHA_TRACE", "0")))
    tmpdir = os.environ.get("MHA_TRACE_DIR") or None
    results = None
    if trace and tmpdir:
        try:
            results, exec_ns = _capture_profile(nc, in_maps, tmpdir)
            _CACHE["last_exec_time_ns"] = exec_ns
        except Exception as e:  # profiling is best-effort
            print(f"profiling unavailable: {type(e).__name__}: {e}")
            results = None
    if results is None:
        results = bass_utils.run_bass_kernel_spmd(
            nc, in_maps, core_ids=list(range(NCORES))).results
        _CACHE.setdefault("last_exec_time_ns", None)

    out = np.empty((B, T, D), np.float32)
    for c in range(NCORES):
        b, g = _core_bg(c)
        o = np.asarray(results[c]["out"], np.float32)
        for qs in range(NSP):
            bb, wq4 = BS[qs], WS[qs] // 4
            out[b, bb + g * wq4: bb + (g + 1) * wq4] = \
                o[bb // 4: bb // 4 + wq4]
    return out


# revision 27
# speedup vs baseline: 1.0478x; 1.0063x over previous
"""Causal multi-head attention (B=2, T=2048, D=1024, H=16) on 8 TRN2 NeuronCores.

Sharding: core c = (batch c%2, head-group c//2); each core owns 4 heads
(256 dims) of one batch. Partial out-projections are summed with one fp16
ReduceScatter per 512-row q-span over each batch's 4 cores; the even/odd
replica groups [[0,2,4,6],[1,3,5,7]] measure ~30us faster than consecutive
grouping on the axon 8-core topology.

Design (vs the 352us baseline this evolved from):
  - software-pipelined score->exp->AV loop (score kt+1 issues before AV kt)
    so the PE does not serialize on the ~630ns exp.
  - causal column trimming: diagonal k-tiles only compute/exp/AV columns
    >= 128*kt - span_base; the partially-masked 128x128 block is the SAME
    upper-triangular pattern for every tile -> one 32KB tri tile replaces
    the 2MB mask load.
  - fp16 partials + one RS per span (half the collective bytes of fp32;
    fp16 keeps quantization at 2^-11 so accuracy is unchanged).
  - RS results copied to the output tensor from the gpsimd queue with a
    one-RS lag: the Tile scheduler orders instructions by cost-model
    readiness, and a collective-gated DMA placed on the scalar/sync queues
    gets scheduled mid-stream and head-of-line blocks exps/partial-writes
    for tens of us when the real RS runs slower than the model.
  - projections of span s+1 and post-processing (normalize/out-proj/RS) of
    span s-1 interleave between the attention heads of span s.
  - per-head softmax denominators from PSUM row 64 (the v-augmentation ones
    column) -> DVE reciprocal_approx_fast (needs an SBUF-resident input:
    from PSUM it returns garbage) -> bf16 rows 0/32 of a persistent [33,512]
    tile -> ONE rank-1 PE matmul per (span, dims-half) with a [33,128] 0/1
    selector broadcasts both heads' 1/den across partitions at once.
  - q/k bias evacuations on ACT via Identity+bias (same act table as Exp,
    so no 1283ns table reloads); yT/v/out evacuations on DVE.

Perf notes: the device clamps the PE clock (power/activity throttle) to
~1.2GHz and later ~0.85GHz, so the kernel is PE-column-bound: projections
98k cols + attention 2x70k + out-proj 33k + bcast 4k. Startup input DMAs
are balanced across all three DMA-capable queues (first matmul ~13-17us
in); the last span's partial writes split across sync+scalar so the final
RS fires ~4us after the last matmul. Measured 214-237us (throttle-state
dependent) vs 352-358us baseline; rel err 5.955e-3; PE idle within the
compute window is <3us.
"""

import os
import numpy as np
import ml_dtypes

BF16 = ml_dtypes.bfloat16
FP16 = np.float16

B, T, D, H = 2, 2048, 1024, 16
HD = D // H                     # 64
NCORES = 8
GROUPS = 4                      # cores per batch (tensor-parallel degree)
HL = H // GROUPS                # heads per core = 4
DL = D // GROUPS                # dims per core = 256
SCALE = HD ** -0.5

# core c = (batch c%2, head-group c//2): the even/odd replica groups measure
# ~30us faster collectives than [[0..3],[4..7]] on the axon 8-core topology
RS_GROUPS = [[0, 2, 4, 6], [1, 3, 5, 7]]


def _core_bg(c):
    return (c % 2, c // 2)


WS = [512, 512, 512, 512]       # q-span widths (sum = T)
BS = [0, 512, 1024, 1536]       # q-span base offsets
NSP = len(WS)

_CACHE = {}


def _build_program():
    import concourse.bass as bass  # noqa: F401  (registers bass machinery)
    import concourse.tile as tile
    from concourse import bacc, mybir

    f32 = mybir.dt.float32
    f32r = mybir.dt.float32r
    bf16 = mybir.dt.bfloat16
    fp16 = mybir.dt.float16
    Exp = mybir.ActivationFunctionType.Exp
    Identity = mybir.ActivationFunctionType.Identity

    nc = bacc.Bacc("TRN2", target_bir_lowering=False, debug=False,
                   num_devices=NCORES)

    xT = nc.dram_tensor("xT", [D, T], bf16, kind="ExternalInput")
    wqT = nc.dram_tensor("wqT", [D, DL], bf16, kind="ExternalInput")
    wkT = nc.dram_tensor("wkT", [D, DL], bf16, kind="ExternalInput")
    wvT = nc.dram_tensor("wvT", [D, DL], bf16, kind="ExternalInput")
    woT = nc.dram_tensor("woT", [DL, D], bf16, kind="ExternalInput")
    bqP = nc.dram_tensor("bqP", [128, 2], f32, kind="ExternalInput")
    bkP = nc.dram_tensor("bkP", [128, 2], f32, kind="ExternalInput")
    bv = nc.dram_tensor("bv", [1, DL], bf16, kind="ExternalInput")
    bo = nc.dram_tensor("bo", [1, D], bf16, kind="ExternalInput")
    trid = nc.dram_tensor("trid", [128, 128], bf16, kind="ExternalInput")
    out_ext = nc.dram_tensor("out", [T // GROUPS, D], fp16,
                             kind="ExternalOutput")

    with tile.TileContext(nc) as tc:
        with tc.tile_pool(name="main", bufs=1) as main, \
             tc.tile_pool(name="rec", bufs=8) as recp, \
             tc.tile_pool(name="at", bufs=4) as atp, \
             tc.tile_pool(name="ob", bufs=3) as obp, \
             tc.tile_pool(name="dram", bufs=1, space="DRAM") as dram, \
             tc.tile_pool(name="sc_ps", bufs=2, space="PSUM") as sc_ps, \
             tc.tile_pool(name="av_ps", bufs=2, space="PSUM") as av_ps, \
             tc.tile_pool(name="pj_ps", bufs=2, space="PSUM") as pj_ps, \
             tc.tile_pool(name="pp_ps", bufs=2, space="PSUM") as pp_ps:

            xt_s = main.tile([128, 8, T], bf16)
            wq_s = main.tile([128, 8, DL], bf16)
            wk_s = main.tile([128, 8, DL], bf16)
            wv_s = main.tile([128, 8, DL], bf16)
            wo_s = main.tile([128, 2, D], bf16)
            qT_s = main.tile([128, 2, T], bf16)
            kT_s = main.tile([128, 2, T], bf16)
            yT_s = main.tile([128, 2, T], bf16)
            v_s = main.tile([128, 16, HL * 65], bf16)
            tri_s = main.tile([128, 128], bf16)
            bq_s = main.tile([128, 2], f32)
            bk_s = main.tile([128, 2], f32)
            bv_bc = main.tile([128, DL], bf16)
            bo_bc = main.tile([128, D], bf16)
            ones_b = main.tile([1, 64], bf16)
            # two-head normalize broadcast: lhsT [33,128] selector places
            # rec row 0 on out partitions 0-63 and rec row 32 on 64-127;
            # rows 1-31 are zeroed so SBUF garbage cannot leak NaNs
            ones2 = main.tile([33, 128], bf16)
            rec2s = [main.tile([33, 512], bf16, name=f"rec2_{i}")
                     for i in range(4)]

            partials = [dram.tile([WS[i], D], fp16, name=f"partial{i}")
                        for i in range(NSP)]
            rs_outs = [dram.tile([WS[i] // GROUPS, D], fp16, name=f"rsout{i}")
                       for i in range(NSP)]

            # ---- input DMAs: the first q-projection group needs all of wq
            # (512KB) plus x[:, :, 0:256] (512KB); balance that 1MB evenly
            # across the three DMA-capable queues (~340KB each)
            wq_r = wqT[:].rearrange("(c p) n -> p c n", p=128)
            xT_r = xT[:].rearrange("(c p) t -> p c t", p=128)
            nc.scalar.dma_start(out=wq_s[:, 0:5, :], in_=wq_r[:, 0:5, :])
            nc.scalar.dma_start(out=bq_s, in_=bqP[:])
            nc.scalar.dma_start(out=bk_s, in_=bkP[:])
            wk_r = wkT[:].rearrange("(c p) n -> p c n", p=128)
            nc.scalar.dma_start(out=wk_s, in_=wk_r)
            nc.gpsimd.dma_start(out=xt_s[:, 2:8, 0:256],
                                in_=xT_r[:, 2:8, 0:256])
            nc.sync.dma_start(out=wq_s[:, 5:8, :], in_=wq_r[:, 5:8, :])
            nc.sync.dma_start(out=xt_s[:, 0:2, 0:256], in_=xT_r[:, 0:2, 0:256])
            nc.sync.dma_start(out=xt_s[:, :, 256:512], in_=xT_r[:, :, 256:512])
            wv_r = wvT[:].rearrange("(c p) n -> p c n", p=128)
            nc.sync.dma_start(out=wv_s, in_=wv_r)
            nc.sync.dma_start(out=bv_bc, in_=bv[:].to_broadcast([128, DL]))
            nc.sync.dma_start(out=tri_s, in_=trid[:])
            nc.sync.dma_start(out=xt_s[:, :, 512:1024], in_=xT_r[:, :, 512:1024])
            wo_r = woT[:].rearrange("(c p) n -> p c n", p=128)
            nc.sync.dma_start(out=wo_s, in_=wo_r)
            nc.sync.dma_start(out=bo_bc, in_=bo[:].to_broadcast([128, D]))
            for lo, hi in ((1024, 1536), (1536, 2048)):
                nc.sync.dma_start(out=xt_s[:, :, lo:hi], in_=xT_r[:, :, lo:hi])

            nc.gpsimd.memset(ones_b, 1.0)
            nc.gpsimd.memset(ones2, 0.0)
            nc.gpsimd.memset(ones2[0:1, 0:64], 1.0)
            nc.gpsimd.memset(ones2[32:33, 64:128], 1.0)
            for r2 in rec2s:
                nc.gpsimd.memset(r2, 0.0)
            nc.vector.memset(v_s, 1.0)   # ones column at index 64 per head

            # ---------------- emission helpers ----------------
            heads_ps = {}   # (qs, h) -> av psum tile awaiting evacuation

            def proj_q(s, w_s, b_s, dst, mcs=(0, 1), nsplit=1):
                bb, ww = BS[s], WS[s]
                for mc in mcs:
                    for sp in range(nsplit):
                        w0 = ww // nsplit
                        lo = bb + sp * w0
                        ps = pj_ps.tile([128, 512], f32, tag="pj")
                        for kc in range(8):
                            nc.tensor.matmul(
                                ps[:, :w0],
                                lhsT=w_s[:, kc, mc * 128:(mc + 1) * 128],
                                rhs=xt_s[:, kc, lo:lo + w0],
                                start=(kc == 0), stop=(kc == 7))
                        nc.scalar.activation(
                            dst[:, mc, lo:lo + w0], ps[:, :w0], Identity,
                            bias=b_s[:, mc:mc + 1])

            def proj_v(s, mts=None):
                if mts is None:
                    mts = range(BS[s] // 128, (BS[s] + WS[s]) // 128)
                for mt in mts:
                    ps = pj_ps.tile([128, 512], f32, tag="pj")
                    for kc in range(8):
                        nc.tensor.matmul(
                            ps[:, :DL],
                            lhsT=xt_s[:, kc, mt * 128:(mt + 1) * 128],
                            rhs=wv_s[:, kc, :],
                            start=(kc == 0), stop=(kc == 7))
                    nc.vector.tensor_add(
                        v_s[:, mt, :].rearrange(
                            "p (h d) -> p h d", d=65)[:, :, 0:64],
                        ps[:, :DL].rearrange("p (h d) -> p h d", d=64),
                        bv_bc.rearrange("p (h d) -> p h d", d=64))

            def attn_head(qs, h):
                bb, ww = BS[qs], WS[qs]
                mc, r0 = divmod(h, 2)
                r0 *= 64
                qv = qT_s[r0:r0 + 64, mc, bb:bb + ww]
                nkt = (bb + ww) // 128
                nfull = bb // 128
                av_t = av_ps.tile([65, 512], f32, tag="av")

                def score(kt):
                    c0 = max(0, 128 * kt - bb)
                    sc_t = sc_ps.tile([128, 512], f32, tag="sc")
                    nc.tensor.matmul(
                        sc_t[:, c0:ww],
                        lhsT=kT_s[r0:r0 + 64, mc, kt * 128:(kt + 1) * 128],
                        rhs=qv[:, c0:ww], start=True, stop=True)
                    return sc_t, c0

                nxt = score(0)
                for kt in range(nkt):
                    sc_t, c0 = nxt
                    if kt + 1 < nkt:
                        nxt = score(kt + 1)  # PE runs ahead of the exp
                    at = atp.tile([128, 512], bf16, tag="at")
                    nc.scalar.activation(at[:, c0:ww], sc_t[:, c0:ww], Exp)
                    if kt >= nfull:  # diagonal tile: mask its 128-col block
                        nc.vector.tensor_mul(
                            at[:, c0:c0 + 128], at[:, c0:c0 + 128], tri_s)
                    nc.tensor.matmul(
                        av_t[:, c0:ww], lhsT=v_s[:, kt, h * 65:(h + 1) * 65],
                        rhs=at[:, c0:ww],
                        start=(kt == 0), stop=(kt == nkt - 1))
                heads_ps[(qs, h)] = av_t

            def evac(qs, h):
                bb, ww = BS[qs], WS[qs]
                mc, r0 = divmod(h, 2)
                r0 *= 64
                av_t = heads_ps.pop((qs, h))
                nc.vector.tensor_copy(
                    yT_s[r0:r0 + 64, mc, bb:bb + ww], av_t[0:64, :ww])
                den = recp.tile([1, 512], f32, tag="den")
                nc.vector.tensor_copy(den[:, :ww], av_t[64:65, :ww])
                rec = recp.tile([1, 512], f32, tag="rec")
                nc.vector.reciprocal_approx_fast(rec[:, :ww], den[:, :ww])
                r2 = rec2s[(qs % 2) * 2 + mc]
                row = (h % 2) * 32
                nc.vector.tensor_copy(r2[row:row + 1, :ww], rec[:, :ww])

            def post_norm(qs):
                bb, ww = BS[qs], WS[qs]
                for mc in range(2):
                    r2 = rec2s[(qs % 2) * 2 + mc]
                    rb = pp_ps.tile([128, 512], f32, tag="pp")
                    nc.tensor.matmul(rb[:, :ww], lhsT=ones2,
                                     rhs=r2[:, :ww],
                                     start=True, stop=True)
                    yv = yT_s[:, mc, bb:bb + ww]
                    nc.vector.tensor_mul(yv, yv, rb[:, :ww])

            def post_qt(qs, lq):
                qt = BS[qs] // 128 + lq
                ob = obp.tile([128, D], fp16, tag="ob")
                for ns in range(2):
                    po = pp_ps.tile([128, 512], f32, tag="pp")
                    for kc in range(2):
                        nc.tensor.matmul(
                            po,
                            lhsT=yT_s[:, kc, qt * 128:(qt + 1) * 128],
                            rhs=wo_s[:, kc, ns * 512:(ns + 1) * 512],
                            start=(kc == 0), stop=(kc == 1))
                    nc.vector.tensor_add(
                        ob[:, ns * 512:(ns + 1) * 512], po,
                        bo_bc[:, ns * 512:(ns + 1) * 512])
                if qs == NSP - 1:
                    # last span: exps are done, so the scalar queue can share
                    # the partial writes; row-halves in parallel across both
                    # queues get the final RS triggered ~2us sooner
                    nc.sync.dma_start(
                        out=partials[qs][lq * 128:lq * 128 + 64, :],
                        in_=ob[0:64, :])
                    nc.scalar.dma_start(
                        out=partials[qs][lq * 128 + 64:(lq + 1) * 128, :],
                        in_=ob[64:128, :])
                else:
                    nc.sync.dma_start(
                        out=partials[qs][lq * 128:(lq + 1) * 128, :], in_=ob)

            def post_rs(qs):
                nc.gpsimd.collective_compute(
                    "ReduceScatter", mybir.AluOpType.add,
                    replica_groups=RS_GROUPS,
                    ins=[partials[qs][:].opt()],
                    outs=[rs_outs[qs][:].opt()])
                # copy of the PREVIOUS span's RS result: it waits on that
                # (long-done) RS only, so it cannot stall this queue; the
                # gpsimd queue carries nothing compute-critical anyway
                if qs >= 1:
                    out_copy(qs - 1)

            def out_copy(qs):
                bb, ww = BS[qs], WS[qs]
                nc.gpsimd.dma_start(out=out_ext[bb // 4:(bb + ww) // 4, :],
                                    in_=rs_outs[qs][:])

            # ---------------- program ----------------
            # span-0 projections ordered so heads 0/1 (dims chunk 0) can
            # start their exps as early as possible
            proj_q(0, wq_s, bq_s, qT_s, mcs=(0,), nsplit=2)
            proj_q(0, wk_s, bk_s, kT_s, mcs=(0,), nsplit=2)
            proj_v(0)
            proj_q(0, wq_s, bq_s, qT_s, mcs=(1,), nsplit=2)
            proj_q(0, wk_s, bk_s, kT_s, mcs=(1,), nsplit=2)

            for qs in range(NSP):
                nqt = WS[qs] // 128
                prev = qs - 1
                if prev >= 0:
                    post_norm(prev)
                attn_head(qs, 0)
                attn_head(qs, 1)
                evac(qs, 0)
                if prev >= 0:
                    for lq in range(0, min(2, WS[prev] // 128)):
                        post_qt(prev, lq)
                if qs + 1 < NSP:
                    proj_q(qs + 1, wq_s, bq_s, qT_s)
                attn_head(qs, 2)
                evac(qs, 1)
                if prev >= 0:
                    for lq in range(2, WS[prev] // 128):
                        post_qt(prev, lq)
                    post_rs(prev)
                if qs + 1 < NSP:
                    proj_q(qs + 1, wk_s, bk_s, kT_s)
                attn_head(qs, 3)
                evac(qs, 2)
                if qs + 1 < NSP:
                    proj_v(qs + 1)
                evac(qs, 3)

            last = NSP - 1
            post_norm(last)
            for lq in range(WS[last] // 128):
                post_qt(last, lq)
            post_rs(last)
            out_copy(NSP - 1)

    nc.compile()
    return nc


def _get_program():
    if "nc" not in _CACHE:
        _CACHE["nc"] = _build_program()
    return _CACHE["nc"]


def _make_in_maps(x, mask, Wq, bq, Wk, bk, Wv, bv, Wo, bo):
    x = np.asarray(x, np.float32)
    Wq = np.asarray(Wq, np.float32)
    Wk = np.asarray(Wk, np.float32)
    Wv = np.asarray(Wv, np.float32)
    Wo = np.asarray(Wo, np.float32)
    bq = np.asarray(bq, np.float32)
    bk = np.asarray(bk, np.float32)
    bv = np.asarray(bv, np.float32)
    bo = np.asarray(bo, np.float32)

    tri = np.triu(np.ones((128, 128), np.float32)).astype(BF16)
    zeros_bo = np.zeros((1, D), np.float32)
    in_maps = []
    xTb = {b: np.ascontiguousarray(x[b].T) for b in range(B)}
    for c in range(NCORES):
        b, g = _core_bg(c)
        sl = slice(g * DL, (g + 1) * DL)
        in_maps.append({
            "xT": xTb[b].astype(BF16),
            "wqT": np.ascontiguousarray((Wq[sl] * SCALE).T).astype(BF16),
            "wkT": np.ascontiguousarray(Wk[sl].T).astype(BF16),
            "wvT": np.ascontiguousarray(Wv[sl].T).astype(BF16),
            "woT": np.ascontiguousarray(Wo[:, sl].T).astype(BF16),
            "bqP": np.ascontiguousarray((bq[sl] * SCALE).reshape(2, 128).T),
            "bkP": np.ascontiguousarray(bk[sl].reshape(2, 128).T),
            "bv": bv[sl].reshape(1, DL).astype(BF16),
            "bo": (bo.reshape(1, D) if g == 0 else zeros_bo).astype(BF16),
            "trid": tri,
        })
    return in_maps


def _capture_profile(nc, in_maps, tmpdir):
    """Run with NTFF capture and process the profile ourselves (the stock
    trace path can't handle the duplicate-executable NTFFs the axon relay
    produces). Returns (results, exec_time_ns|None)."""
    import glob
    import json
    import re
    import subprocess
    from trn_agent_boot.trn_boot import _ntff_profile_via_ctypes
    from concourse import bass2jax

    hook = _ntff_profile_via_ctypes("/opt/axon/libaxon_pjrt.so")
    if hook is None:
        raise RuntimeError("libaxon_pjrt.so lacks NTFF profile symbols")
    os.makedirs(tmpdir, exist_ok=True)
    with hook(tmpdir, [0]):
        results = bass2jax.run_bass_via_pjrt(nc, in_maps, n_cores=NCORES)

    ntffs = glob.glob(os.path.join(tmpdir, "*_body*-device*.ntff"))
    best, best_id = None, -1
    for f in ntffs:
        m = re.search(r"executable(\d+)-device000000", f)
        if m and int(m.group(1)) > best_id:
            best_id, best = int(m.group(1)), f
    if best is None:
        raise RuntimeError(f"no NTFF produced in {tmpdir}")
    neff = re.sub(r"-device\d+-execution-\d+\.ntff$", ".neff", best)
    out_json = os.path.join(tmpdir, "prof.json")
    subprocess.check_call(
        ["neuron-profile", "view", "--ignore-nc-buf-usage", "-s", best,
         "-n", neff, "--output-format=json", f"--output-file={out_json}"],
        cwd=tmpdir)
    summary = json.load(open(out_json))["summary"][0]
    return results, int(summary["total_time"] * 1e9)


def kernel(x, mask, Wq, bq, Wk, bk, Wv, bv, Wo, bo):
    from concourse import bass_utils

    in_maps = _make_in_maps(x, mask, Wq, bq, Wk, bk, Wv, bv, Wo, bo)
    nc = _get_program()

    trace = bool(int(os.environ.get("MHA_TRACE", "0")))
    tmpdir = os.environ.get("MHA_TRACE_DIR") or None
    results = None
    if trace and tmpdir:
        try:
            results, exec_ns = _capture_profile(nc, in_maps, tmpdir)
            _CACHE["last_exec_time_ns"] = exec_ns
        except Exception as e:  # profiling is best-effort
            print(f"profiling unavailable: {type(e).__name__}: {e}")
            results = None
    if results is None:
        results = bass_utils.run_bass_kernel_spmd(
            nc, in_maps, core_ids=list(range(NCORES))).results
        _CACHE.setdefault("last_exec_time_ns", None)

    out = np.empty((B, T, D), np.float32)
    for c in range(NCORES):
        b, g = _core_bg(c)
        o = np.asarray(results[c]["out"], np.float32)
        for qs in range(NSP):
            bb, wq4 = BS[qs], WS[qs] // 4
            out[b, bb + g * wq4: bb + (g + 1) * wq4] = \
                o[bb // 4: bb // 4 + wq4]
    return out


# revision 29
# speedup vs baseline: 1.0657x; 1.0170x over previous
"""Causal multi-head attention (B=2, T=2048, D=1024, H=16) on 8 TRN2 NeuronCores.

Sharding: core c = (batch c%2, head-group c//2); each core owns 4 heads
(256 dims) of one batch. Partial out-projections are summed with one fp16
ReduceScatter per 512-row q-span over each batch's 4 cores; the even/odd
replica groups [[0,2,4,6],[1,3,5,7]] measure ~30us faster than consecutive
grouping on the axon 8-core topology.

Design (vs the 352us baseline this evolved from):
  - software-pipelined score->exp->AV loop (score kt+1 issues before AV kt)
    so the PE does not serialize on the ~630ns exp.
  - causal column trimming: diagonal k-tiles only compute/exp/AV columns
    >= 128*kt - span_base; the partially-masked 128x128 block is the SAME
    upper-triangular pattern for every tile -> one 32KB tri tile replaces
    the 2MB mask load.
  - fp16 partials + one RS per span (half the collective bytes of fp32;
    fp16 keeps quantization at 2^-11 so accuracy is unchanged).
  - RS results copied to the output tensor from the gpsimd queue with a
    one-RS lag: the Tile scheduler orders instructions by cost-model
    readiness, and a collective-gated DMA placed on the scalar/sync queues
    gets scheduled mid-stream and head-of-line blocks exps/partial-writes
    for tens of us when the real RS runs slower than the model.
  - projections of span s+1 and post-processing (normalize/out-proj/RS) of
    span s-1 interleave between the attention heads of span s.
  - per-head softmax denominators from PSUM row 64 (the v-augmentation ones
    column) -> DVE reciprocal_approx_fast (needs an SBUF-resident input:
    from PSUM it returns garbage) -> bf16 rows 0/32 of a persistent [33,512]
    tile -> ONE rank-1 PE matmul per (span, dims-half) with a [33,128] 0/1
    selector broadcasts both heads' 1/den across partitions at once.
  - q/k bias evacuations on ACT via Identity+bias (same act table as Exp,
    so no 1283ns table reloads); yT/v/out evacuations on DVE.

Perf notes: the device clamps the PE clock (power/activity throttle) to
~1.2GHz and later ~0.85GHz, so the kernel is PE-column-bound: projections
98k cols + attention 2x70k + out-proj 33k + bcast 4k. Startup input DMAs
are balanced across all three DMA-capable queues (first matmul ~13-17us
in); the last span's partial writes split across sync+scalar so the final
RS fires ~4us after the last matmul. Measured 214-237us (throttle-state
dependent) vs 352-358us baseline; rel err 5.955e-3; PE idle within the
compute window is <3us.
"""

import os
import numpy as np
import ml_dtypes

BF16 = ml_dtypes.bfloat16
FP16 = np.float16

B, T, D, H = 2, 2048, 1024, 16
HD = D // H                     # 64
NCORES = 8
GROUPS = 4                      # cores per batch (tensor-parallel degree)
HL = H // GROUPS                # heads per core = 4
DL = D // GROUPS                # dims per core = 256
SCALE = HD ** -0.5

# core c = (batch c%2, head-group c//2): the even/odd replica groups measure
# ~30us faster collectives than [[0..3],[4..7]] on the axon 8-core topology
RS_GROUPS = [[0, 2, 4, 6], [1, 3, 5, 7]]


def _core_bg(c):
    return (c % 2, c // 2)


WS = [512, 512, 512, 512]       # q-span widths (sum = T)
BS = [0, 512, 1024, 1536]       # q-span base offsets
NSP = len(WS)

_CACHE = {}


def _build_program():
    import concourse.bass as bass  # noqa: F401  (registers bass machinery)
    import concourse.tile as tile
    from concourse import bacc, mybir

    f32 = mybir.dt.float32
    f32r = mybir.dt.float32r
    bf16 = mybir.dt.bfloat16
    fp16 = mybir.dt.float16
    Exp = mybir.ActivationFunctionType.Exp
    Identity = mybir.ActivationFunctionType.Identity

    nc = bacc.Bacc("TRN2", target_bir_lowering=False, debug=False,
                   num_devices=NCORES)

    xT = nc.dram_tensor("xT", [D, T], bf16, kind="ExternalInput")
    wqT = nc.dram_tensor("wqT", [D, DL], bf16, kind="ExternalInput")
    wkT = nc.dram_tensor("wkT", [D, DL], bf16, kind="ExternalInput")
    wvT = nc.dram_tensor("wvT", [D, DL], bf16, kind="ExternalInput")
    woT = nc.dram_tensor("woT", [DL, D], bf16, kind="ExternalInput")
    bqP = nc.dram_tensor("bqP", [128, 2], f32, kind="ExternalInput")
    bkP = nc.dram_tensor("bkP", [128, 2], f32, kind="ExternalInput")
    bv = nc.dram_tensor("bv", [1, DL], bf16, kind="ExternalInput")
    bo = nc.dram_tensor("bo", [1, D], bf16, kind="ExternalInput")
    trid = nc.dram_tensor("trid", [128, 128], bf16, kind="ExternalInput")
    out_ext = nc.dram_tensor("out", [T // GROUPS, D], fp16,
                             kind="ExternalOutput")

    with tile.TileContext(nc) as tc:
        with tc.tile_pool(name="main", bufs=1) as main, \
             tc.tile_pool(name="rec", bufs=8) as recp, \
             tc.tile_pool(name="at", bufs=4) as atp, \
             tc.tile_pool(name="ob", bufs=3) as obp, \
             tc.tile_pool(name="dram", bufs=1, space="DRAM") as dram, \
             tc.tile_pool(name="sc_ps", bufs=2, space="PSUM") as sc_ps, \
             tc.tile_pool(name="av_ps", bufs=2, space="PSUM") as av_ps, \
             tc.tile_pool(name="pj_ps", bufs=2, space="PSUM") as pj_ps, \
             tc.tile_pool(name="pp_ps", bufs=2, space="PSUM") as pp_ps:

            xt_s = main.tile([128, 8, T], bf16)
            wq_s = main.tile([128, 8, DL], bf16)
            wk_s = main.tile([128, 8, DL], bf16)
            wv_s = main.tile([128, 8, DL], bf16)
            wo_s = main.tile([128, 2, D], bf16)
            qT_s = main.tile([128, 2, T], bf16)
            kT_s = main.tile([128, 2, T], bf16)
            yT_s = main.tile([128, 2, T], bf16)
            v_s = main.tile([128, 16, HL * 65], bf16)
            tri_s = main.tile([128, 128], bf16)
            bq_s = main.tile([128, 2], f32)
            bk_s = main.tile([128, 2], f32)
            bv_bc = main.tile([128, DL], bf16)
            bo_bc = main.tile([128, D], bf16)
            ones_b = main.tile([1, 64], bf16)
            # two-head normalize broadcast: lhsT [33,128] selector places
            # rec row 0 on out partitions 0-63 and rec row 32 on 64-127;
            # rows 1-31 are zeroed so SBUF garbage cannot leak NaNs
            ones2 = main.tile([33, 128], bf16)
            rec2s = [main.tile([33, 512], bf16, name=f"rec2_{i}")
                     for i in range(4)]

            partials = [dram.tile([WS[i], D], fp16, name=f"partial{i}")
                        for i in range(NSP)]
            rs_outs = [dram.tile([WS[i] // GROUPS, D], fp16, name=f"rsout{i}")
                       for i in range(NSP)]

            # ---- input DMAs: the first q-projection group needs all of wq
            # (512KB) plus x[:, :, 0:256] (512KB); balance that 1MB evenly
            # across the three DMA-capable queues (~340KB each)
            wq_r = wqT[:].rearrange("(c p) n -> p c n", p=128)
            xT_r = xT[:].rearrange("(c p) t -> p c t", p=128)
            nc.scalar.dma_start(out=wq_s[:, 0:5, :], in_=wq_r[:, 0:5, :])
            nc.scalar.dma_start(out=bq_s, in_=bqP[:])
            nc.scalar.dma_start(out=bk_s, in_=bkP[:])
            wk_r = wkT[:].rearrange("(c p) n -> p c n", p=128)
            nc.scalar.dma_start(out=wk_s, in_=wk_r)
            nc.gpsimd.dma_start(out=xt_s[:, 2:8, 0:256],
                                in_=xT_r[:, 2:8, 0:256])
            nc.sync.dma_start(out=wq_s[:, 5:8, :], in_=wq_r[:, 5:8, :])
            nc.sync.dma_start(out=xt_s[:, 0:2, 0:256], in_=xT_r[:, 0:2, 0:256])
            nc.sync.dma_start(out=xt_s[:, :, 256:512], in_=xT_r[:, :, 256:512])
            wv_r = wvT[:].rearrange("(c p) n -> p c n", p=128)
            nc.sync.dma_start(out=wv_s, in_=wv_r)
            nc.sync.dma_start(out=bv_bc, in_=bv[:].to_broadcast([128, DL]))
            nc.sync.dma_start(out=tri_s, in_=trid[:])
            nc.sync.dma_start(out=xt_s[:, :, 512:1024], in_=xT_r[:, :, 512:1024])
            wo_r = woT[:].rearrange("(c p) n -> p c n", p=128)
            nc.sync.dma_start(out=wo_s, in_=wo_r)
            nc.sync.dma_start(out=bo_bc, in_=bo[:].to_broadcast([128, D]))
            for lo, hi in ((1024, 1536), (1536, 2048)):
                nc.sync.dma_start(out=xt_s[:, :, lo:hi], in_=xT_r[:, :, lo:hi])

            nc.gpsimd.memset(ones_b, 1.0)
            nc.gpsimd.memset(ones2, 0.0)
            nc.gpsimd.memset(ones2[0:1, 0:64], 1.0)
            nc.gpsimd.memset(ones2[32:33, 64:128], 1.0)
            for r2 in rec2s:
                nc.gpsimd.memset(r2, 0.0)
            nc.vector.memset(v_s, 1.0)   # ones column at index 64 per head

            # ---------------- emission helpers ----------------
            heads_ps = {}   # (qs, h) -> av psum tile awaiting evacuation

            def proj_q(s, w_s, b_s, dst, mcs=(0, 1), nsplit=1):
                bb, ww = BS[s], WS[s]
                for mc in mcs:
                    for sp in range(nsplit):
                        w0 = ww // nsplit
                        lo = bb + sp * w0
                        ps = pj_ps.tile([128, 512], f32, tag="pj")
                        for kc in range(8):
                            nc.tensor.matmul(
                                ps[:, :w0],
                                lhsT=w_s[:, kc, mc * 128:(mc + 1) * 128],
                                rhs=xt_s[:, kc, lo:lo + w0],
                                start=(kc == 0), stop=(kc == 7))
                        nc.scalar.activation(
                            dst[:, mc, lo:lo + w0], ps[:, :w0], Identity,
                            bias=b_s[:, mc:mc + 1])

            def proj_v(s, mts=None):
                if mts is None:
                    mts = range(BS[s] // 128, (BS[s] + WS[s]) // 128)
                for mt in mts:
                    ps = pj_ps.tile([128, 512], f32, tag="pj")
                    for kc in range(8):
                        nc.tensor.matmul(
                            ps[:, :DL],
                            lhsT=xt_s[:, kc, mt * 128:(mt + 1) * 128],
                            rhs=wv_s[:, kc, :],
                            start=(kc == 0), stop=(kc == 7))
                    nc.vector.tensor_add(
                        v_s[:, mt, :].rearrange(
                            "p (h d) -> p h d", d=65)[:, :, 0:64],
                        ps[:, :DL].rearrange("p (h d) -> p h d", d=64),
                        bv_bc.rearrange("p (h d) -> p h d", d=64))

            def attn_head(qs, h):
                bb, ww = BS[qs], WS[qs]
                mc, r0 = divmod(h, 2)
                r0 *= 64
                qv = qT_s[r0:r0 + 64, mc, bb:bb + ww]
                nkt = (bb + ww) // 128
                nfull = bb // 128
                av_t = av_ps.tile([65, 512], f32, tag="av")

                def score(kt):
                    c0 = max(0, 128 * kt - bb)
                    sc_t = sc_ps.tile([128, 512], f32, tag="sc")
                    nc.tensor.matmul(
                        sc_t[:, c0:ww],
                        lhsT=kT_s[r0:r0 + 64, mc, kt * 128:(kt + 1) * 128],
                        rhs=qv[:, c0:ww], start=True, stop=True)
                    return sc_t, c0

                nxt = score(0)
                for kt in range(nkt):
                    sc_t, c0 = nxt
                    if kt + 1 < nkt:
                        nxt = score(kt + 1)  # PE runs ahead of the exp
                    at = atp.tile([128, 512], bf16, tag="at")
                    nc.scalar.activation(at[:, c0:ww], sc_t[:, c0:ww], Exp)
                    if kt >= nfull:  # diagonal tile: mask its 128-col block
                        nc.vector.tensor_mul(
                            at[:, c0:c0 + 128], at[:, c0:c0 + 128], tri_s)
                    nc.tensor.matmul(
                        av_t[:, c0:ww], lhsT=v_s[:, kt, h * 65:(h + 1) * 65],
                        rhs=at[:, c0:ww],
                        start=(kt == 0), stop=(kt == nkt - 1))
                heads_ps[(qs, h)] = av_t

            def evac(qs, h):
                bb, ww = BS[qs], WS[qs]
                mc, r0 = divmod(h, 2)
                r0 *= 64
                av_t = heads_ps.pop((qs, h))
                nc.vector.tensor_copy(
                    yT_s[r0:r0 + 64, mc, bb:bb + ww], av_t[0:64, :ww])
                den = recp.tile([1, 512], f32, tag="den")
                nc.vector.tensor_copy(den[:, :ww], av_t[64:65, :ww])
                rec = recp.tile([1, 512], f32, tag="rec")
                nc.vector.reciprocal_approx_fast(rec[:, :ww], den[:, :ww])
                r2 = rec2s[(qs % 2) * 2 + mc]
                row = (h % 2) * 32
                nc.vector.tensor_copy(r2[row:row + 1, :ww], rec[:, :ww])

            def post_norm(qs):
                bb, ww = BS[qs], WS[qs]
                for mc in range(2):
                    r2 = rec2s[(qs % 2) * 2 + mc]
                    rb = pp_ps.tile([128, 512], f32, tag="pp")
                    nc.tensor.matmul(rb[:, :ww], lhsT=ones2,
                                     rhs=r2[:, :ww],
                                     start=True, stop=True)
                    yv = yT_s[:, mc, bb:bb + ww]
                    nc.vector.tensor_mul(yv, yv, rb[:, :ww])

            def post_qt(qs, lq):
                qt = BS[qs] // 128 + lq
                ob = obp.tile([128, D], fp16, tag="ob")
                for ns in range(2):
                    po = pp_ps.tile([128, 512], f32, tag="pp")
                    for kc in range(2):
                        nc.tensor.matmul(
                            po,
                            lhsT=yT_s[:, kc, qt * 128:(qt + 1) * 128],
                            rhs=wo_s[:, kc, ns * 512:(ns + 1) * 512],
                            start=(kc == 0), stop=(kc == 1))
                    nc.vector.tensor_add(
                        ob[:, ns * 512:(ns + 1) * 512], po,
                        bo_bc[:, ns * 512:(ns + 1) * 512])
                if qs == NSP - 1:
                    # last span: exps are done, so the scalar queue can share
                    # the partial writes; row-halves in parallel across both
                    # queues get the final RS triggered ~2us sooner
                    nc.sync.dma_start(
                        out=partials[qs][lq * 128:lq * 128 + 64, :],
                        in_=ob[0:64, :])
                    nc.scalar.dma_start(
                        out=partials[qs][lq * 128 + 64:(lq + 1) * 128, :],
                        in_=ob[64:128, :])
                else:
                    nc.sync.dma_start(
                        out=partials[qs][lq * 128:(lq + 1) * 128, :], in_=ob)

            def post_rs(qs):
                nc.gpsimd.collective_compute(
                    "ReduceScatter", mybir.AluOpType.add,
                    replica_groups=RS_GROUPS,
                    ins=[partials[qs][:].opt()],
                    outs=[rs_outs[qs][:].opt()])
                # copy of the PREVIOUS span's RS result: it waits on that
                # (long-done) RS only, so it cannot stall this queue; the
                # gpsimd queue carries nothing compute-critical anyway
                if qs >= 1:
                    out_copy(qs - 1)

            def out_copy(qs):
                bb, ww = BS[qs], WS[qs]
                nc.gpsimd.dma_start(out=out_ext[bb // 4:(bb + ww) // 4, :],
                                    in_=rs_outs[qs][:])

            # ---------------- program ----------------
            # span-0 projections ordered so heads 0/1 (dims chunk 0) can
            # start their exps as early as possible
            proj_q(0, wq_s, bq_s, qT_s, mcs=(0,), nsplit=2)
            proj_q(0, wk_s, bk_s, kT_s, mcs=(0,), nsplit=2)
            proj_v(0)
            proj_q(0, wq_s, bq_s, qT_s, mcs=(1,), nsplit=2)
            proj_q(0, wk_s, bk_s, kT_s, mcs=(1,), nsplit=2)

            for qs in range(NSP):
                nqt = WS[qs] // 128
                prev = qs - 1
                if prev >= 0:
                    post_norm(prev)
                attn_head(qs, 0)
                attn_head(qs, 1)
                evac(qs, 0)
                if prev >= 0:
                    for lq in range(0, min(2, WS[prev] // 128)):
                        post_qt(prev, lq)
                if qs + 1 < NSP:
                    proj_q(qs + 1, wq_s, bq_s, qT_s)
                attn_head(qs, 2)
                evac(qs, 1)
                if prev >= 0:
                    for lq in range(2, WS[prev] // 128):
                        post_qt(prev, lq)
                    post_rs(prev)
                if qs + 1 < NSP:
                    proj_q(qs + 1, wk_s, bk_s, kT_s)
                attn_head(qs, 3)
                evac(qs, 2)
                if qs + 1 < NSP:
                    proj_v(qs + 1)
                evac(qs, 3)

            last = NSP - 1
            post_norm(last)
            for lq in range(WS[last] // 128):
                post_qt(last, lq)
            post_rs(last)
            out_copy(NSP - 1)

    nc.compile()
    return nc


def _get_program():
    if "nc" not in _CACHE:
        _CACHE["nc"] = _build_program()
    return _CACHE["nc"]


def _make_in_maps(x, mask, Wq, bq, Wk, bk, Wv, bv, Wo, bo):
    x = np.asarray(x, np.float32)
    Wq = np.asarray(Wq, np.float32)
    Wk = np.asarray(Wk, np.float32)
    Wv = np.asarray(Wv, np.float32)
    Wo = np.asarray(Wo, np.float32)
    bq = np.asarray(bq, np.float32)
    bk = np.asarray(bk, np.float32)
    bv = np.asarray(bv, np.float32)
    bo = np.asarray(bo, np.float32)

    tri = np.triu(np.ones((128, 128), np.float32)).astype(BF16)
    zeros_bo = np.zeros((1, D), np.float32)
    in_maps = []
    xTb = {b: np.ascontiguousarray(x[b].T) for b in range(B)}
    for c in range(NCORES):
        b, g = _core_bg(c)
        sl = slice(g * DL, (g + 1) * DL)
        in_maps.append({
            "xT": xTb[b].astype(BF16),
            "wqT": np.ascontiguousarray((Wq[sl] * SCALE).T).astype(BF16),
            "wkT": np.ascontiguousarray(Wk[sl].T).astype(BF16),
            "wvT": np.ascontiguousarray(Wv[sl].T).astype(BF16),
            "woT": np.ascontiguousarray(Wo[:, sl].T).astype(BF16),
            "bqP": np.ascontiguousarray((bq[sl] * SCALE).reshape(2, 128).T),
            "bkP": np.ascontiguousarray(bk[sl].reshape(2, 128).T),
            "bv": bv[sl].reshape(1, DL).astype(BF16),
            "bo": (bo.reshape(1, D) if g == 0 else zeros_bo).astype(BF16),
            "trid": tri,
        })
    return in_maps


def _capture_profile(nc, in_maps, tmpdir):
    """Run with NTFF capture and process the profile ourselves (the stock
    trace path can't handle the duplicate-executable NTFFs the axon relay
    produces). Returns (results, exec_time_ns|None)."""
    import glob
    import json
    import re
    import subprocess
    from trn_agent_boot.trn_boot import _ntff_profile_via_ctypes
    from concourse import bass2jax

    hook = _ntff_profile_via_ctypes("/opt/axon/libaxon_pjrt.so")
    if hook is None:
        raise RuntimeError("libaxon_pjrt.so lacks NTFF profile symbols")
    os.makedirs(tmpdir, exist_ok=True)
    with hook(tmpdir, [0]):
        results = bass2jax.run_bass_via_pjrt(nc, in_maps, n_cores=NCORES)

    ntffs = glob.glob(os.path.join(tmpdir, "*_body*-device*.ntff"))
    best, best_id = None, -1
    for f in ntffs:
        m = re.search(r"executable(\d+)-device000000", f)
        if m and int(m.group(1)) > best_id:
            best_id, best = int(m.group(1)), f
    if best is None:
        raise RuntimeError(f"no NTFF produced in {tmpdir}")
    neff = re.sub(r"-device\d+-execution-\d+\.ntff$", ".neff", best)
    out_json = os.path.join(tmpdir, "prof.json")
    subprocess.check_call(
        ["neuron-profile", "view", "--ignore-nc-buf-usage", "-s", best,
         "-n", neff, "--output-format=json", f"--output-file={out_json}"],
        cwd=tmpdir)
    summary = json.load(open(out_json))["summary"][0]
    return results, int(summary["total_time"] * 1e9)


def kernel(x, mask, Wq, bq, Wk, bk, Wv, bv, Wo, bo):
    from concourse import bass_utils

    in_maps = _make_in_maps(x, mask, Wq, bq, Wk, bk, Wv, bv, Wo, bo)
    nc = _get_program()

    trace = bool(int(os.environ.get("MHA_TRACE", "0")))
    tmpdir = os.environ.get("MHA_TRACE_DIR") or None
    results = None
    if trace and tmpdir:
        try:
            results, exec_ns = _capture_profile(nc, in_maps, tmpdir)
            _CACHE["last_exec_time_ns"] = exec_ns
        except Exception as e:  # profiling is best-effort
            print(f"profiling unavailable: {type(e).__name__}: {e}")
            results = None
    if results is None:
        results = bass_utils.run_bass_kernel_spmd(
            nc, in_maps, core_ids=list(range(NCORES))).results
        _CACHE.setdefault("last_exec_time_ns", None)

    out = np.empty((B, T, D), np.float32)
    for c in range(NCORES):
        b, g = _core_bg(c)
        o = np.asarray(results[c]["out"], np.float32)
        for qs in range(NSP):
            bb, wq4 = BS[qs], WS[qs] // 4
            out[b, bb + g * wq4: bb + (g + 1) * wq4] = \
                o[bb // 4: bb // 4 + wq4]
    return out
